# revision 83
# baseline (speedup 1.0000x reference)
"""Trainium2 Bass kernel for nn_Cross_Attention_Fourier.

Math: with ortho-normalized FFTs, fft2 -> q@k^H -> ifft2 collapses exactly:
  ifft2(fft2(q) @ conj(fft2(k))^T) = (q @ k^T) @ J,  J: j -> (-j) mod n
so the block is plain attention with scores |q@k^T|, softmax/sqrt(d), applied
to row-flipped v.  No complex arithmetic.  The post-softmax 1/sqrt(d) scale
is absorbed by the downstream global mean/std normalization and dropped.

Sharding (8 cores): core c -> sample b = c//2, query-token half (c%2)*512.
Each core computes LN+QKV for its slice (keys/values for the whole sample),
8 heads of attention; the FiLM t-vector is sharded 8-way and AllReduced
early (hidden under attention), the sample-global mean/std needs a second
tiny [4,2] AllReduce after attention; then output projection + feed-forward
on its 512 tokens.

Layout: activations are dim-major (feature dim on partitions, tokens free).
LayerNorm is FOLDED into the projections: project raw x, add a rank-1
correction (-colsum(W) (x) mean-row) via a K=1 matmul.  The per-token
inv-std scaling is applied where cheapest: q at psum evacuation (DVE,
against a DMA-broadcast inv row), k folded into the per-partition `scale`
of the Exp activation (k-tokens are partitions of S^T), v folded into the
ACT evacuation scale.  |S| is one pass (bitwise_and 0x7fffffff on int32
view) split between DVE and GpSimd; the softmax denominator is folded into
the attn@v matmul as a 65th all-ones stationary column.  Reciprocals use
the fast approx DVE op.  Matmuls run as float32r (full PE rate at N>=256).
"""

import numpy as np

import concourse.bass as bass
import concourse.bacc as bacc
import concourse.mybir as mybir
import concourse.tile as tile
from concourse.bass_utils import run_bass_kernel_spmd

AF = mybir.ActivationFunctionType
ALU = mybir.AluOpType
F32 = mybir.dt.float32
F32R = mybir.dt.float32r
I32 = mybir.dt.int32

N_CORES = 8
B = 4
NT = 1024          # tokens (keys)
TQ = 512           # query tokens per core
D = 512            # model dim
H = 8              # heads
DH = 64            # head dim
DC = 4             # dim chunks of 128
KT = 8             # key-token tiles of 128
E2 = 1024          # 2*D (FiLM width)
NEL = float(NT * D)

def _abs_on_act(hp, kt):
    """score pairs whose |.| runs as ACT Abs (2nd ACT pass) instead of a
    DVE bitwise-and; balances DVE vs ACT occupancy in the attention phase."""
    return kt == 5 or (hp % 2 == 0 and kt == 2)


def f32(ap):
    return ap.bitcast(F32)


def _build_nc(gelu_mode="hw", has_bias=False):
    global _GELU_FUNC
    _GELU_FUNC = AF.Gelu if gelu_mode == "hw" else AF.Tanh
    nc = bacc.Bacc("TRN2", target_bir_lowering=False, debug=False,
                   num_devices=N_CORES)

    def din(name, shape):
        return nc.dram_tensor(name, shape, F32, kind="ExternalInput").ap()

    t = dict(
        xq=din("xq", [D, TQ]),
        xkv=din("xkv", [D, NT]),
        xv=din("xv", [D, NT]),
        wq=din("wq", [D, D]),
        wk=din("wk", [D, D]),
        wv=din("wv", [D, D]),
        wo=din("wo", [D, D]),
        m1=din("m1", [D, D]),
        m2=din("m2", [D, D]),
        nws=din("nws", [1, 4 * D]),      # -colsum rows: q,k,v,m1
        w1e=din("w1e", [D, 128]),
        w2e=din("w2e", [128, E2]),
        emb=din("emb", [D, B]),
        b2e=din("b2e", [1, E2]),
        smalls=din("smalls", [128, 266]),
        pbias=din("pbias", [1, 4 * D]),  # bq,bk,bv,b1 rows (bias mode)
    )
    t["out"] = nc.dram_tensor("out", [D, TQ], F32, kind="ExternalOutput").ap()
    t["has_bias"] = has_bias

    with tile.TileContext(nc) as tc:
        _emit(nc, tc, t)
    nc.compile()
    return nc


def _emit(nc, tc, t):
    xq, xkv, xv = t["xq"], t["xkv"], t["xv"]
    wq, wk, wv, wo, m1, m2 = t["wq"], t["wk"], t["wv"], t["wo"], t["m1"], t["m2"]
    w1e, w2e, emb = t["w1e"], t["w2e"], t["emb"]
    nws, b2e = t["nws"], t["b2e"]
    smalls, pbias, out = t["smalls"], t["pbias"], t["out"]
    has_bias = t["has_bias"]
    LP = dict(reason="f32r output is fp32 bits")

    from contextlib import ExitStack
    ctx = ExitStack()
    with ctx:
        cpool = ctx.enter_context(tc.tile_pool(name="const", bufs=1))
        rowpool = ctx.enter_context(tc.tile_pool(name="rows", bufs=1))
        outpool = ctx.enter_context(tc.tile_pool(name="outp", bufs=1))
        dpool = ctx.enter_context(tc.tile_pool(name="dram", bufs=1, space="DRAM"))

        # ---- constants: one packed DMA (dispatch is ~800ns per DMA) ----
        smt = rowpool.tile([128, 266], F32R, tag="smalls")
        nc.sync.dma_start(smt[:], smalls[:].bitcast(F32R))
        ones_col = smt[:, 0:1]
        ones_col_f = f32(smt[:, 0:1])
        onesf = f32(smt[0:1, 0:128])
        b1e_sb = f32(smt[:, 128:129])
        bias_sb = {"bo": f32(smt[:, 129:133]), "b2": f32(smt[:, 133:137])}
        sel_sb = f32(smt[0:B, 137:138])
        selr_sb = smt[0:B, 137:138]
        sel128_sb = f32(smt[0:B, 138:266])

        nws_row = rowpool.tile([1, 4 * D], F32R, tag="nws_row")
        nc.sync.dma_start(nws_row[:], nws[:].bitcast(F32R))
        nws_sb = [nws_row[0:1, r * D:(r + 1) * D] for r in range(4)]
        pb_sb = []
        if has_bias:
            pb_row = rowpool.tile([1, 4 * D], F32R, tag="pb_row")
            nc.sync.dma_start(pb_row[:], pbias[:].bitcast(F32R))
            pb_sb = [pb_row[0:1, r * D:(r + 1) * D] for r in range(4)]

        def srt(w, name):
            return rowpool.tile([1, w], F32, tag="scratchrow", bufs=2,
                                name=name)[:, 0:w]

        def scw(name):
            return rowpool.tile([128, 1], F32, tag="scw", bufs=8, name=name)[:]

        def load_cols(src, n, tag, pool, dt=F32R):
            tiles = []
            for j in range(n):
                tl = pool.tile([128, src.shape[1]], dt, tag=f"{tag}{j}",
                               name=f"{tag}{j}")
                nc.sync.dma_start(tl[:], src[j * 128:(j + 1) * 128, :].bitcast(dt))
                tiles.append(tl)
            return tiles

        def load_packed(src, n, w, tag, pool, dt=F32R, eng=None):
            """one DMA for an [n*128, w] dram tensor -> [128, n*w] sbuf tile;
            returns the n chunk-slice APs."""
            big = pool.tile([128, n * w], dt, tag=tag, name=tag)
            (eng or nc.sync).dma_start(
                big[:].rearrange("p (j t) -> p j t", j=n),
                src.rearrange("(j p) t -> p j t", p=128).bitcast(dt))
            return [big[:, j * w:(j + 1) * w] for j in range(n)]

        w1e_t = load_packed(w1e, DC, 128, "w1e", cpool, dt=F32)
        w2e_t = load_cols(w2e, 1, "w2e", cpool)
        emb_t = load_packed(emb, DC, B, "emb", cpool, dt=F32)

        ar1_in_d = dpool.tile([B, E2], F32, tag="ar1_in_d")
        ar1_out_d = dpool.tile([B, E2], F32, tag="ar1_out_d")
        ar2_in_d = dpool.tile([1, 2], F32, tag="ar2_in_d")
        ar2_out_d = dpool.tile([1, 2], F32, tag="ar2_out_d")

        # ---- FiLM partial + early collective #1 ------------------------
        with tc.tile_pool(name="psF", bufs=1, space="PSUM") as psF:
            ps_f = psF.tile([128, B], F32, tag="ftp_f")
            for j in range(DC):
                nc.tensor.matmul(ps_f[:], w1e_t[j][:], emb_t[j][:],
                                 start=(j == 0), stop=(j == DC - 1))
            xb_sb = rowpool.tile([128, B], F32, tag="xb")
            nc.scalar.activation(xb_sb[:], ps_f[:], AF.Identity, bias=b1e_sb[:])
            sg_sb = rowpool.tile([128, B], F32, tag="sg")
            nc.scalar.activation(sg_sb[:], xb_sb[:], AF.Sigmoid)
            silu_sb = rowpool.tile([128, B], F32R, tag="silu")
            nc.vector.tensor_tensor(silu_sb[:], xb_sb[:], sg_sb[:], op=ALU.mult)
            ps_t = psF.tile([B, E2], F32, tag="ftp_t")
            for a in range(2):
                nc.tensor.matmul(ps_t[:, a * 512:(a + 1) * 512], silu_sb[:],
                                 w2e_t[0][:, a * 512:(a + 1) * 512],
                                 start=True, stop=True)
            ar1_in = rowpool.tile([B, E2], F32, tag="ar1in")
            nc.vector.tensor_copy(ar1_in[:], ps_t[:])
            nc.sync.dma_start(ar1_in_d[:], ar1_in[:])
        nc.gpsimd.collective_compute(
            "AllReduce", ALU.add, replica_groups=[list(range(N_CORES))],
            ins=[ar1_in_d.opt()], outs=[ar1_out_d.opt()])
        ar1_sb = rowpool.tile([B, E2], F32R, tag="ar1sb")
        nc.sync.dma_start(ar1_sb[:], ar1_out_d[:].bitcast(F32R))

        attnpool = ctx.enter_context(tc.tile_pool(name="attn", bufs=1))
        qTp = [attnpool.tile([128, TQ], F32R, tag=f"qTp{p}", name=f"qTp{p}")
               for p in range(DC)]
        kTp = [attnpool.tile([128, NT], F32R, tag=f"kTp{p}", name=f"kTp{p}")
               for p in range(DC)]
        vtbig = attnpool.tile([128, KT * H * 65], F32R, tag="vtbig")
        vt = [vtbig[:, t_ * H * 65:(t_ + 1) * H * 65] for t_ in range(KT)]
        nc.scalar.dma_start(
            vtbig[:].rearrange("p (ti h x) -> p ti h x", ti=KT, h=H)
            [:, :, :, DH:DH + 1],
            smalls[:, 0:64].rearrange("p (a b) -> p a b", a=KT)
            .unsqueeze(3).bitcast(F32R))
        # per-partition inv-std columns for k tokens (exp scale) / v tokens
        ivkc = rowpool.tile([128, KT], F32, tag="ivkc")
        ivc_v = rowpool.tile([128, KT], F32, tag="ivc_v")
        ivb_q = rowpool.tile([128, TQ], F32, tag="ivb_q")

        tailrows = ctx.enter_context(tc.tile_pool(name="tailrows", bufs=1))
        b2e_sb = tailrows.tile([1, E2], F32, tag="b2e")
        nc.sync.dma_start(b2e_sb[:], b2e[:])
        mean_t = tailrows.tile([1, 512], F32, tag="mean_t")
        std_t = tailrows.tile([1, 512], F32, tag="std_t")
        mtc = tailrows.tile([128, DC], F32R, tag="mtc")
        stc = tailrows.tile([128, DC], F32R, tag="stc")

        # x inputs next on the DMA queues: they gate the first stats/proj
        # compute; the big weight set follows them
        raw_stack = ExitStack()
        rawpool = raw_stack.enter_context(tc.tile_pool(name="rawp", bufs=1))

        def load_raws(src, T, tag):
            return load_packed(src, DC, T, tag, rawpool)

        raw_kv = load_raws(xkv, NT, "rawk")
        raw_q = load_raws(xq, TQ, "rawq")

        # ---- LN stats + folded projections -----------------------------
        with tc.tile_pool(name="lnsq", bufs=2) as lnsq, \
             tc.tile_pool(name="lnrows", bufs=1) as lnrows, \
             tc.tile_pool(name="psLN", bufs=1, space="PSUM") as psLN:

            def ln_stats_head(raws, T, mtag):
                """emits squares (ACT) + sum-stats (PE) + mean row;
                returns (sqs, mrow)."""
                sqs = []
                for j in range(DC):
                    sq = lnsq.tile([128, T], F32R, tag="sq", name="sq")
                    nc.scalar.activation(sq[:], f32(raws[j][:]), AF.Square)
                    sqs.append(sq)
                ps_s = []
                # sum-stats depend only on the raw loads, so the PE is not
                # stalled on the ACT squares
                for a in range(T // 512):
                    sl = slice(a * 512, (a + 1) * 512)
                    ps = psLN.tile([1, 512], F32, tag="lnS", bufs=2,
                                   name=f"ps_s{a}")
                    for j in range(DC):
                        nc.tensor.matmul(ps[:], ones_col[:],
                                         raws[j][:, sl], start=(j == 0),
                                         stop=(j == DC - 1),
                                         skip_group_check=True)
                    ps_s.append(ps)
                # in the centered (no-bias) path mrow is consumed right away,
                # so a per-width rotating tag suffices
                mtag_ = f"mrow_{mtag}" if has_bias else f"mrow{T}"
                mrow = lnrows.tile([1, T], F32R, tag=mtag_, bufs=1,
                                   name=f"mrow_{mtag}")
                for a in range(T // 512):
                    nc.vector.tensor_scalar_mul(
                        mrow[:, a * 512:(a + 1) * 512], ps_s[a][:], 1.0 / D)
                return sqs, mrow

            def ln_stats_sq(sqs, T):
                """emits the sum-of-squares stat matmuls."""
                ps_q = []
                for a in range(T // 512):
                    sl = slice(a * 512, (a + 1) * 512)
                    pq = psLN.tile([1, 512], F32, tag="lnQ", bufs=2,
                                   name=f"ps_q{a}")
                    for j in range(DC):
                        nc.tensor.matmul(pq[:], ones_col[:],
                                         sqs[j][:, sl], start=(j == 0),
                                         stop=(j == DC - 1),
                                         skip_group_check=True)
                    ps_q.append(pq)
                return ps_q

            def ln_stats_tail(ps_q, mrow, T, mtag):
                """finishes var -> sd -> inv from the stat psums."""
                var = srt(T, "var")
                for a in range(T // 512):
                    nc.vector.tensor_scalar(var[:, a * 512:(a + 1) * 512],
                                            ps_q[a][:], 1.0 / D, 1e-5,
                                            op0=ALU.mult, op1=ALU.add)
                msq = srt(T, "msq")
                nc.scalar.activation(msq, f32(mrow[:]), AF.Square)
                nc.vector.tensor_tensor(var, var, msq, op=ALU.subtract)
                if has_bias:
                    sd = lnrows.tile([1, T], F32R, tag=f"sd_{mtag}",
                                     name=f"sd_{mtag}")[:]
                else:
                    sd = lnrows.tile([1, T], F32R, tag="sdrot", bufs=1,
                                     name="sd")[:, 0:T]
                nc.scalar.activation(sd, var, AF.Sqrt)
                inv = lnrows.tile([1, T], F32, tag=f"inv_{mtag}",
                                  name=f"inv_{mtag}")[:, 0:T]
                nc.vector.reciprocal_approx_fast(inv, f32(sd))
                return inv, sd

            def inv_to_cols(inv, dst, nt_):
                """transpose a [1, nt_*128] inv row into [128, nt_] columns."""
                for ti in range(nt_):
                    pp = psLN.tile([128, 1024], F32, tag="pk1", bufs=2,
                                   name="pp")
                    nc.tensor.transpose(pp[:, 0:1],
                                        inv[0:1, ti * 128:(ti + 1) * 128],
                                        onesf[0:1, 0:1])
                    nc.vector.tensor_copy(dst[:, ti:ti + 1], pp[:, 0:1])

            def center(raws, mrow, T, mtag):
                """x <- x - mean (per token), replacing the rank-1 matmul."""
                mb = lnrows.tile([128, T], F32R, tag=f"mb{T}", bufs=1,
                                 name=f"mb_{mtag}")
                nc.gpsimd.partition_broadcast(mb[:], mrow[:])
                for j in range(DC):
                    nc.vector.tensor_tensor(raws[j][:], f32(raws[j][:]),
                                            f32(mb[:]), op=ALU.subtract)

            with tc.tile_pool(name="zw1", bufs=1) as zw1:
                # weight loads dispatch on the (idle) ACT hwdge queue so
                # they don't queue behind the 5MB of raw-input DMAs
                wk_t = load_packed(wk, DC, D, "wk", zw1, eng=nc.scalar)
                wq_t = load_packed(wq, DC, D, "wq", zw1, eng=nc.scalar)
                raw_v = load_raws(xv, NT, "rawv")
                wv_t = load_packed(wv, DC, D, "wv", zw1, eng=nc.scalar)

                # ---- k (unnormalized; inv_k folded into exp scale) ----
                sq_kv, mrow_kv = ln_stats_head(raw_kv, NT, "kv")
                sd_kv = None
                if has_bias:
                    psq_kv = ln_stats_sq(sq_kv, NT)
                    inv_kv, sd_kv = ln_stats_tail(psq_kv, mrow_kv, NT, "kv")
                else:
                    center(raw_kv, mrow_kv, NT, "kv")
                for mi in range(DC):
                    msl = slice(mi * 128, (mi + 1) * 128)
                    pp = psLN.tile([128, 1024], F32, tag="pk1", bufs=2,
                                   name="pp")
                    for a in range(NT // 512):
                        sl = slice(a * 512, (a + 1) * 512)
                        for j in range(DC):
                            nc.tensor.matmul(pp[:, sl], wk_t[j][:, msl],
                                             raw_kv[j][:, sl],
                                             start=(j == 0),
                                             stop=(not has_bias
                                                   and j == DC - 1),
                                             skip_group_check=True)
                        if has_bias:
                            nc.tensor.matmul(pp[:, sl], nws_sb[1][:, msl],
                                             mrow_kv[:, sl], start=False,
                                             stop=False, skip_group_check=True)
                            nc.tensor.matmul(pp[:, sl], pb_sb[1][:, msl],
                                             sd_kv[:, sl], start=False,
                                             stop=True, skip_group_check=True)
                    nc.scalar.activation(kTp[mi][:], pp[:], AF.Identity)

                # ---- q ----
                sq_q, mrow_q = ln_stats_head(raw_q, TQ, "q")
                if not has_bias:
                    center(raw_q, mrow_q, TQ, "q")
                    psq_kv = ln_stats_sq(sq_kv, NT)
                    inv_kv, sd_kv = ln_stats_tail(psq_kv, mrow_kv, NT, "kv")
                inv_to_cols(inv_kv, ivkc, KT)
                psq_q = ln_stats_sq(sq_q, TQ)
                inv_q, sd_q = ln_stats_tail(psq_q, mrow_q, TQ, "q")
                nc.gpsimd.partition_broadcast(ivb_q[:], inv_q)
                for mi in range(DC):
                    msl = slice(mi * 128, (mi + 1) * 128)
                    pp = psLN.tile([128, 1024], F32, tag="pk1", bufs=2,
                                   name="pp")
                    for j in range(DC):
                        nc.tensor.matmul(pp[:, 0:512], wq_t[j][:, msl],
                                         raw_q[j][:], start=(j == 0),
                                         stop=(not has_bias and j == DC - 1),
                                         skip_group_check=True)
                    if has_bias:
                        nc.tensor.matmul(pp[:, 0:512], nws_sb[0][:, msl],
                                         mrow_q[:], start=False, stop=False,
                                         skip_group_check=True)
                        nc.tensor.matmul(pp[:, 0:512], pb_sb[0][:, msl],
                                         sd_q[:], start=False, stop=True,
                                         skip_group_check=True)
                    nc.vector.tensor_tensor(qTp[mi][:], pp[:, 0:512],
                                            ivb_q[:], op=ALU.mult)

                # ---- v (inv_v folded into ACT evacuation scale) ----
                sq_v, mrow_v = ln_stats_head(raw_v, NT, "v")
                if not has_bias:
                    center(raw_v, mrow_v, NT, "v")
                psq_v = ln_stats_sq(sq_v, NT)
                inv_v, sd_v = ln_stats_tail(psq_v, mrow_v, NT, "v")
                inv_to_cols(inv_v, ivc_v, KT)
                for ti in range(KT):
                    tsl = slice(ti * 128, (ti + 1) * 128)
                    pv = psLN.tile([128, 1024], F32, tag="pk1", bufs=2,
                                   name="pv")
                    for j in range(DC):
                        nc.tensor.matmul(pv[:, 0:512], raw_v[j][:, tsl],
                                         wv_t[j][:], start=(j == 0),
                                         stop=(not has_bias and j == DC - 1),
                                         skip_group_check=True)
                    if has_bias:
                        nc.tensor.matmul(pv[:, 0:512], mrow_v[:, tsl],
                                         nws_sb[2][:], start=False, stop=False,
                                         skip_group_check=True)
                        nc.tensor.matmul(pv[:, 0:512], sd_v[:, tsl], pb_sb[2][:],
                                         start=False, stop=True,
                                         skip_group_check=True)
                    vw = vt[ti].rearrange("p (h x) -> p h x", h=H)
                    nc.scalar.activation(
                        vw[:, :, 0:DH],
                        pv[:, 0:512].rearrange("p (h x) -> p h x", h=H),
                        AF.Identity, scale=ivc_v[:, ti:ti + 1])

                # FiLM rows (needs AR1, which has landed by now): mean_t/std_t
                # rows + their per-partition column transposes
                psel = psLN.tile([128, 1024], F32, tag="pk1", bufs=2,
                                 name="psel")
                for a in range(2):
                    asl = slice(a * 512, (a + 1) * 512)
                    nc.tensor.matmul(psel[0:1, asl], selr_sb[:],
                                     ar1_sb[:, asl], start=True, stop=True,
                                     skip_group_check=True)
                nc.vector.tensor_tensor(mean_t[:], psel[0:1, 0:512],
                                        b2e_sb[:, 0:512], op=ALU.add)
                nc.vector.tensor_tensor(std_t[:], psel[0:1, 512:E2],
                                        b2e_sb[:, 512:E2], op=ALU.add)
                for j in range(DC):
                    jsl = slice(j * 128, (j + 1) * 128)
                    p1 = psLN.tile([128, 1024], F32, tag="pk1", bufs=2,
                                   name="p1")
                    nc.tensor.transpose(p1[:, 0:1], mean_t[0:1, jsl],
                                        onesf[0:1, 0:1])
                    nc.vector.tensor_copy(mtc[:, j:j + 1], p1[:, 0:1])
                    p2 = psLN.tile([128, 1024], F32, tag="pk1", bufs=2,
                                   name="p2")
                    nc.tensor.transpose(p2[:, 0:1], std_t[0:1, jsl],
                                        onesf[0:1, 0:1])
                    nc.vector.tensor_copy(stc[:, j:j + 1], p2[:, 0:1])

        raw_stack.close()   # raws are dead; free their SBUF for attention

        # weights for the tail; DMA overlaps attention
        wpool2 = ctx.enter_context(tc.tile_pool(name="w2", bufs=1))
        wo_t = load_packed(wo, DC, D, "wo", wpool2, eng=nc.scalar)
        m1_t = load_packed(m1, DC, D, "m1", wpool2, eng=nc.scalar)
        m2_t = load_packed(m2, DC, D, "m2", wpool2, eng=nc.scalar)
        wot_s = [wpool2.tile([128, D], F32R, tag=f"wos{j}", name=f"wos{j}")
                 for j in range(DC)]

        # ---- attention (two heads per score pair) ----------------------
        outT = [outpool.tile([128, TQ], F32R, tag=f"outT{j}", name=f"outT{j}")
                for j in range(DC)]

        # per-pair row sums (cols 0-3) and square sums (cols 4-7) of outT,
        # accumulated for free by the normalizing DVE passes
        acc = tailrows.tile([128, 2 * DC], F32, tag="acc")

        with tc.tile_pool(name="ep", bufs=1) as epool, \
             tc.tile_pool(name="gsq1", bufs=2) as gsq1, \
             tc.tile_pool(name="psA", bufs=1, space="PSUM") as psA:

            def make_finisher(hp, po0, po1):
                def fin():
                    for po_, half in ((po0, 0), (po1, 1)):
                        recd = srt(TQ, "recd")
                        nc.vector.tensor_copy(recd, po_[64:65, :])
                        rec = rowpool.tile([1, TQ], F32, tag="rec", bufs=2,
                                           name="rec")
                        nc.vector.reciprocal_approx_fast(rec[:], recd)
                        rb_sb = epool.tile([64, TQ], F32, tag="rbsb", bufs=2,
                                           name="rb_sb")
                        nc.gpsimd.partition_broadcast(rb_sb[:], rec[:])
                        nc.vector.scalar_tensor_tensor(
                            outT[hp][half * 64:(half + 1) * 64, :],
                            po_[0:64, :], 1.0, rb_sb[:],
                            op0=ALU.bypass, op1=ALU.mult,
                            accum_out=acc[half * 64:(half + 1) * 64,
                                          hp:hp + 1])
                    sqd = gsq1.tile([128, TQ], F32R, tag="gsq", name="sqd")
                    nc.vector.scalar_tensor_tensor(
                        sqd[:], f32(outT[hp][:]), 1.0, f32(outT[hp][:]),
                        op0=ALU.bypass, op1=ALU.mult,
                        accum_out=acc[:, DC + hp:DC + hp + 1])
                return fin

            finish = []
            for hp in range(DC):
                h0, h1 = 2 * hp, 2 * hp + 1
                po0 = psA.tile([65, TQ], F32, tag="po", bufs=2, name="po0")
                po1 = psA.tile([65, TQ], F32, tag="po", bufs=2, name="po1")
                exs = []
                po_emitted = 0

                def emit_po(kt):
                    nc.tensor.matmul(po0[:], vt[kt][:, h0 * 65:(h0 + 1) * 65],
                                     exs[kt][:, 0:TQ], start=(kt == 0),
                                     stop=(kt == KT - 1),
                                     skip_group_check=True)
                    nc.tensor.matmul(po1[:], vt[kt][:, h1 * 65:(h1 + 1) * 65],
                                     exs[kt][:, TQ:2 * TQ], start=(kt == 0),
                                     stop=(kt == KT - 1),
                                     skip_group_check=True)

                for kt in range(KT):
                    ksl = slice(kt * 128, (kt + 1) * 128)
                    pst = psA.tile([128, 2 * TQ], F32, tag="stps", bufs=3,
                                   name="pst")
                    nc.tensor.matmul(pst[:, 0:TQ], kTp[hp][0:64, ksl],
                                     qTp[hp][0:64, :], start=True, stop=True,
                                     skip_group_check=True)
                    nc.tensor.matmul(pst[:, TQ:2 * TQ], kTp[hp][64:128, ksl],
                                     qTp[hp][64:128, :], start=True, stop=True,
                                     skip_group_check=True)
                    ex = epool.tile([128, 2 * TQ], F32R, tag="ex", bufs=3,
                                    name="ex")
                    if _abs_on_act(hp, kt):
                        abf = epool.tile([128, 2 * TQ], F32, tag="abf", bufs=1,
                                         name="abf")
                        nc.scalar.activation(abf[:], pst[:], AF.Abs,
                                             scale=ivkc[:, kt:kt + 1])
                        nc.scalar.activation(ex[:], abf[:], AF.Exp)
                    else:
                        ab = epool.tile([128, 2 * TQ], I32, tag="ab", bufs=2,
                                        name="ab")
                        nc.vector.tensor_scalar(ab[:], pst[:].bitcast(I32),
                                                0x7FFFFFFF, None,
                                                op0=ALU.bitwise_and)
                        nc.scalar.activation(ex[:], ab[:].bitcast(F32), AF.Exp,
                                             scale=ivkc[:, kt:kt + 1])
                    exs.append(ex)
                    if kt == 1 and finish:
                        # previous pair's normalize, off the critical DVE path
                        finish.pop(0)()
                    if kt >= 2:
                        emit_po(po_emitted)
                        po_emitted += 1
                while po_emitted < KT:
                    emit_po(po_emitted)
                    po_emitted += 1
                finish.append(make_finisher(hp, po0, po1))
                if hp == 2:
                    # std_t-scaled output weights for the pre-AR2 projection
                    for j in range(DC):
                        nc.vector.tensor_scalar(wot_s[j][:], f32(wo_t[j][:]),
                                                f32(stc[:, j:j + 1]), None,
                                                op0=ALU.mult)
            while finish:
                finish.pop(0)()
            # totals over the accumulated per-partition sums
            gsp = psA.tile([128, 2 * TQ], F32, tag="stps", bufs=3, name="gsp")
            nc.tensor.matmul(gsp[0:1, 0:2 * DC], ones_col_f[:], acc[:],
                             start=True, stop=True, skip_group_check=True)
            # pairwise AR2 sums the two half-sample totals directly; no
            # per-sample table or sel mask needed
            srow = rowpool.tile([1, 2], F32, tag="srow")
            nc.vector.reduce_sum(srow[:, 0:1], gsp[0:1, 0:DC],
                                 axis=mybir.AxisListType.X)
            nc.vector.reduce_sum(srow[:, 1:2], gsp[0:1, DC:2 * DC],
                                 axis=mybir.AxisListType.X)
            nc.sync.dma_start(ar2_in_d[:], srow[:])

        # only the two cores sharing a sample need each other's partial sums
        # (sel128 masks the other rows), so pairwise groups suffice
        nc.gpsimd.collective_compute(
            "AllReduce", ALU.add,
            replica_groups=[[2 * i, 2 * i + 1] for i in range(N_CORES // 2)],
            ins=[ar2_in_d.opt()], outs=[ar2_out_d.opt()])
        ar2_sb = rowpool.tile([1, 2], F32, tag="ar2sb")
        nc.sync.dma_start(ar2_sb[:], ar2_out_d[:])

        # ---- tail: out-proj (pre-AR2 on scaled weights), MLP ------------
        with tc.tile_pool(name="mlpp", bufs=1) as mlppool, \
             tc.tile_pool(name="gsqp", bufs=2) as gsqp, \
             tc.tile_pool(name="psP", bufs=1, space="PSUM") as psP:
            # y_raw = (std_t . wo)^T @ outT  -- independent of AR2, so these
            # 16 matmuls hide the collective's latency
            pyr = [psP.tile([128, TQ], F32, tag=f"pyr{mo}", name=f"pyr{mo}")
                   for mo in range(DC)]
            for mo in range(DC):
                for j in range(DC):
                    nc.tensor.matmul(pyr[mo][:],
                                     wot_s[j][:, mo * 128:(mo + 1) * 128],
                                     outT[j][:], start=(j == 0),
                                     stop=(j == DC - 1))
            # wo^T mean_t and wo^T std_t rows (pre-AR2; evacuated so the pk2
            # ring frees before the post-AR2 chain needs it)
            pm_sb = tailrows.tile([1, 512], F32, tag="pm_sb")
            ps_sb = tailrows.tile([1, 512], F32, tag="ps_sb")
            for row_sb, cols in ((pm_sb, mtc), (ps_sb, stc)):
                pr = psP.tile([128, TQ], F32, tag="pk2", bufs=2, name="pr")
                for j in range(DC):
                    nc.tensor.matmul(pr[0:1, :], cols[:, j:j + 1], wo_t[j][:],
                                     start=(j == 0), stop=(j == DC - 1),
                                     skip_group_check=True)
                nc.vector.tensor_copy(row_sb[:], pr[0:1, :])

            # unscaled y_raw into SBUF, then mlp1 partials on it -- all
            # pre-AR2 so the PE chews through these during the collective
            yr = [mlppool.tile([128, TQ], F32R, tag="yr", bufs=4,
                               name=f"yr{mo}") for mo in range(DC)]
            for mo in range(DC):
                nc.scalar.activation(yr[mo][:], pyr[mo][:], AF.Identity)
            P = []
            if not has_bias:
                for mo in range(DC):
                    msl = slice(mo * 128, (mo + 1) * 128)
                    pp = psP.tile([128, TQ], F32, tag=f"pyr{mo}", name="P")
                    for j in range(DC):
                        nc.tensor.matmul(pp[:], m1_t[j][:, msl], yr[j][:],
                                         start=(j == 0), stop=False,
                                         skip_group_check=True)
                    P.append(pp)

            ps_st = psP.tile([128, TQ], F32, tag="pk2", bufs=2, name="ps_st")
            nc.tensor.matmul(ps_st[:, 0:2], onesf, ar2_sb[:],
                             start=True, stop=True, skip_group_check=True)
            mu = scw("mu")
            nc.vector.tensor_scalar_mul(mu, ps_st[:, 0:1], 1.0 / NEL)
            smu = scw("smu")
            nc.vector.tensor_tensor(smu, ps_st[:, 0:1], mu, op=ALU.mult)
            var1 = scw("var1")
            nc.vector.tensor_tensor(var1, ps_st[:, 1:2], smu, op=ALU.subtract)
            var1s = scw("var1s")
            nc.vector.tensor_scalar_mul(var1s, var1, 1.0 / (NEL - 1.0))
            sd_g = scw("sd_g")
            nc.scalar.activation(sd_g, var1s, AF.Sqrt)
            inv_sd = scw("inv_sd")
            nc.vector.reciprocal(inv_sd, sd_g)
            nmu = scw("nmu")
            nc.vector.tensor_scalar_mul(nmu, mu, -1.0)
            nms = scw("nms")
            nc.vector.tensor_tensor(nms, nmu, inv_sd, op=ALU.mult)  # -mu/sd
            # bias row bt_row = wo^T mean_t - (mu/sd) wo^T std_t, then
            # transpose to per-partition columns and add bo
            btr = srt(TQ, "btr")
            nc.vector.tensor_scalar(btr, ps_sb[:], nms[0:1, 0:1], None,
                                    op0=ALU.mult)
            nc.vector.tensor_tensor(btr, btr, pm_sb[:], op=ALU.add)
            bt = tailrows.tile([128, DC], F32R, tag="bt")
            for mo in range(DC):
                pt_ = psP.tile([128, TQ], F32, tag="pk2", bufs=2, name="pt_")
                nc.tensor.transpose(pt_[:, 0:1],
                                    btr[0:1, mo * 128:(mo + 1) * 128],
                                    onesf[0:1, 0:1])
                nc.vector.tensor_tensor(bt[:, mo:mo + 1], pt_[:, 0:1],
                                        bias_sb["bo"][:, mo:mo + 1],
                                        op=ALU.add)

            def proj(win, rhs, bias_tile, func, outtiles):
                for mo in range(DC):
                    pp = psP.tile([128, TQ], F32, tag="pk2", bufs=2, name="pp")
                    for j in range(DC):
                        nc.tensor.matmul(pp[:], win[j][:, mo * 128:(mo + 1) * 128],
                                         rhs[j][:], start=(j == 0),
                                         stop=(j == DC - 1))
                    nc.scalar.activation(outtiles[mo][:], pp[:], func,
                                         bias=bias_tile[:, mo:mo + 1])

            y = [mlppool.tile([128, TQ], F32R, tag=f"y{j}", name=f"y{j}")
                 for j in range(DC)]
            for mo in range(DC):
                nc.scalar.activation(y[mo][:], f32(yr[mo][:]), AF.Identity,
                                     bias=f32(bt[:, mo:mo + 1]),
                                     scale=inv_sd)

            # mlp layernorm stats
            ps_s2 = psP.tile([1, TQ], F32, tag="prow2", bufs=2, name="ps_s2")
            ps_q2 = psP.tile([1, TQ], F32, tag="prow2", bufs=2, name="ps_q2")
            for j in range(DC):
                sq = gsqp.tile([128, TQ], F32R, tag="gsq", name="sq")
                nc.vector.tensor_tensor(sq[:], f32(y[j][:]), f32(y[j][:]),
                                        op=ALU.mult)
                nc.tensor.matmul(ps_s2[:], ones_col[:], y[j][:],
                                 start=(j == 0), stop=(j == DC - 1),
                                 skip_group_check=True)
                nc.tensor.matmul(ps_q2[:], ones_col[:], sq[:],
                                 start=(j == 0), stop=(j == DC - 1),
                                 skip_group_check=True)
            m2row = tailrows.tile([1, TQ], F32R, tag="m2row")
            nc.vector.tensor_scalar_mul(m2row[:], ps_s2[:], 1.0 / D)
            if not has_bias:
                # mean_y pre-divided by the global scale c=1/sd: the open
                # mlp1 psum groups accumulate (1/c)*rank1 so the later *c
                # rescale reproduces m1^T y + nws3 (x) mean_y exactly
                m2rs = tailrows.tile([1, TQ], F32R, tag="m2rs")
                nc.vector.tensor_scalar(m2rs[:], f32(m2row[:]),
                                        sd_g[0:1, 0:1], None, op0=ALU.mult)
            var2 = srt(TQ, "var2")
            nc.vector.tensor_scalar(var2, ps_q2[:], 1.0 / D, 1e-5,
                                    op0=ALU.mult, op1=ALU.add)
            msq2 = srt(TQ, "msq2")
            nc.scalar.activation(msq2, f32(m2row[:]), AF.Square)
            nc.vector.tensor_tensor(var2, var2, msq2, op=ALU.subtract)
            sd2 = tailrows.tile([1, TQ], F32R, tag="sd2")
            nc.scalar.activation(sd2[:], var2, AF.Sqrt)
            inv2 = tailrows.tile([1, TQ], F32, tag="inv2")
            nc.vector.reciprocal_approx_fast(inv2[:], f32(sd2[:]))
            i2b = tailrows.tile([128, TQ], F32, tag="i2b")
            nc.gpsimd.partition_broadcast(i2b[:], inv2[:])

            # mlp1 with folded LN: gelu(inv2 * (m1^T y - m1sum (x) m2row))
            g = [mlppool.tile([128, TQ], F32R, tag="mlpbuf", bufs=4,
                              name=f"g{j}") for j in range(DC)]
            if not has_bias:
                # cc = m1^T bt columns (the bias part of y through mlp1)
                pcc = psP.tile([128, TQ], F32, tag="pk2", bufs=2, name="pcc")
                for j in range(DC):
                    nc.tensor.matmul(pcc[0:1, :], bt[:, j:j + 1], m1_t[j][:],
                                     start=(j == 0), stop=(j == DC - 1),
                                     skip_group_check=True)
                ccr = srt(TQ, "ccr")
                nc.vector.tensor_copy(ccr, pcc[0:1, :])
                cc = tailrows.tile([128, DC], F32, tag="cc")
                for mo in range(DC):
                    pt2 = psP.tile([128, TQ], F32, tag="pk2", bufs=2,
                                   name="pt2")
                    nc.tensor.transpose(pt2[:, 0:1],
                                        ccr[0:1, mo * 128:(mo + 1) * 128],
                                        onesf[0:1, 0:1])
                    nc.vector.tensor_copy(cc[:, mo:mo + 1], pt2[:, 0:1])
                for mo in range(DC):
                    msl = slice(mo * 128, (mo + 1) * 128)
                    nc.tensor.matmul(P[mo][:], nws_sb[3][:, msl], m2rs[:],
                                     start=False, stop=True,
                                     skip_group_check=True)
                    zin = gsqp.tile([128, TQ], F32R, tag="gin", bufs=2, name="zin")
                    nc.vector.tensor_scalar(zin[:], P[mo][:], inv_sd,
                                            cc[:, mo:mo + 1], op0=ALU.mult,
                                            op1=ALU.add)
                    gin = gsqp.tile([128, TQ], F32R, tag="gin", bufs=2, name="gin")
                    nc.vector.tensor_tensor(gin[:], f32(zin[:]), i2b[:],
                                            op=ALU.mult)
                    nc.scalar.activation(g[mo][:], f32(gin[:]), _GELU_FUNC)
            else:
                for mo in range(DC):
                    msl = slice(mo * 128, (mo + 1) * 128)
                    pp = psP.tile([128, TQ], F32, tag="pk2", bufs=2, name="pp")
                    for j in range(DC):
                        nc.tensor.matmul(pp[:], m1_t[j][:, msl], y[j][:],
                                         start=(j == 0), stop=False)
                    nc.tensor.matmul(pp[:], nws_sb[3][:, msl], m2row[:],
                                     start=False, stop=False)
                    nc.tensor.matmul(pp[:], pb_sb[3][:, msl], sd2[:],
                                     start=False, stop=True)
                    gin = gsqp.tile([128, TQ], F32R, tag="gin", bufs=2, name="gin")
                    nc.vector.tensor_tensor(gin[:], pp[:], i2b[:], op=ALU.mult)
                    nc.scalar.activation(g[mo][:], f32(gin[:]), _GELU_FUNC)

            yf = [mlppool.tile([128, TQ], F32, tag="mlpbuf2", bufs=4,
                               name=f"yf{j}") for j in range(DC)]
            proj(m2_t, g, bias_sb["b2"], AF.Identity, yf)
            for j in range(DC):
                nc.sync.dma_start(out[j * 128:(j + 1) * 128, :], yf[j][:])


_NC_CACHE = {}
_GELU_FUNC = AF.Gelu


def _get_nc(gelu_mode="hw", has_bias=False):
    key = (gelu_mode, has_bias)
    if key not in _NC_CACHE:
        _NC_CACHE[key] = _build_nc(gelu_mode, has_bias)
    return _NC_CACHE[key]


def _prep_in_maps(inputs):
    f = lambda k: np.ascontiguousarray(np.asarray(inputs[k], dtype=np.float32))
    diff, con, temb = f("diff_features"), f("con_features"), f("time_emb")
    g_d, b_d = f("ln_diff_g"), f("ln_diff_b")
    g_c, b_c = f("ln_con_g"), f("ln_con_b")
    wq_, wk_, wv_ = f("wq"), f("wk"), f("wv")
    wo_, bo_ = f("w_out"), f("b_out")
    w1e_, b1e_, w2e_, b2e_ = f("w_emd1"), f("b_emd1"), f("w_emd2"), f("b_emd2")
    gm, bm = f("mlp_ln_g"), f("mlp_ln_b")
    m1_, mb1_, m2_, mb2_ = f("mlp_w1"), f("mlp_b1"), f("mlp_w2"), f("mlp_b2")

    wq_f = g_d[:, None] * wq_
    wk_f = g_c[:, None] * wk_
    wv_f = g_c[:, None] * wv_
    bq_v = b_d @ wq_
    bk_v = b_c @ wk_
    bv_v = b_c @ wv_
    m1_f = gm[:, None] * m1_
    mb1_f = mb1_ + bm @ m1_
    has_bias = bool(np.any(bq_v) or np.any(bk_v) or np.any(bv_v)
                    or np.any(mb1_f))
    nws = -np.stack([wq_f.sum(0), wk_f.sum(0), wv_f.sum(0), m1_f.sum(0)])
    pbias = np.stack([bq_v, bk_v, bv_v, mb1_f])
    flip = (-np.arange(NT)) % NT

    def br(v):
        return np.ascontiguousarray(v.reshape(DC, 128).T)

    common = {
        "wq": wq_f, "wk": wk_f, "wv": wv_f, "wo": wo_,
        "m1": m1_f, "m2": m2_,
        "nws": nws.reshape(1, 4 * D),
        "pbias": pbias.reshape(1, 4 * D),
        "emb": np.ascontiguousarray(temb.T),
        "b2e": b2e_.reshape(1, E2),
    }
    in_maps = []
    for c in range(N_CORES):
        b, off = c // 2, (c % 2) * TQ
        sm = np.zeros((128, 266), np.float32)
        sm[:, 0:128] = 1.0
        sm[:, 128] = b1e_[c * 128:(c + 1) * 128]
        sm[:, 129:133] = br(bo_)
        sm[:, 133:137] = br(mb2_)
        sm[b, 137] = 1.0
        sm[b, 138:266] = 1.0
        m = dict(common)
        m.update({
            "xq": np.ascontiguousarray(diff[b, off:off + TQ].T),
            "xkv": np.ascontiguousarray(con[b].T),
            "xv": np.ascontiguousarray(con[b][flip].T),
            "w1e": np.ascontiguousarray(w1e_[:, c * 128:(c + 1) * 128]),
            "w2e": np.ascontiguousarray(w2e_[c * 128:(c + 1) * 128, :]),
            "smalls": sm,
        })
        in_maps.append({k: np.ascontiguousarray(v.astype(np.float32))
                        for k, v in m.items()})
    return in_maps, has_bias


def _assemble(results):
    outp = np.empty((B, NT, D), np.float32)
    for c in range(N_CORES):
        b, off = c // 2, (c % 2) * TQ
        outp[b, off:off + TQ, :] = results[c]["out"].T
    return outp


def kernel(**inputs):
    in_maps, has_bias = _prep_in_maps(inputs)
    nc = _get_nc("hw", has_bias)
    res = run_bass_kernel_spmd(nc, in_maps, core_ids=list(range(N_CORES)))
    return _assemble(res.results)
